# revision 24
# baseline (speedup 1.0000x reference)
"""Trainium2 Bass kernel for a 2-layer LSTM classifier forward pass + softmax CE loss.

Model (see harness reference): B=2048, T=80, C=80 classes, E=8 embed, H=256 hidden.
  x = embedding[features]                             [B, T, 8]
  2-layer BasicLSTMCell scan over T (gates i,j,f,o; forget_bias=1.0)
  pred = h1_last @ Wd + bd                            [B, 80]
  loss = mean_b( -sum_c labels*log_softmax(pred) )    scalar

Sharding: pure data parallel. Batch 2048 -> 8 cores x 256 rows. All parameters
replicated. Each core computes a partial sum of per-sample losses; host sums the
8 partials and divides by B (the only host-side arithmetic).

Device design (per core), "transposed state" form so no transposes are needed:
  - states hT: [hidden (2x128 part-chunks), batch 256 free] fp8 tiles [128,2,256]
    (fp8e4m3 so the recurrence matmuls run in DoubleRow perf mode); cell state
    cT kept in fp16 [128, 512] so the DVE elementwise chain runs in 2x mode.
  - gates: [gate-dim chunk (128 part), batch 256 free] x 8 chunks side by side
    in a [128, 2048] PSUM tile per layer; device gate order [f, i, o, j] means
    each gate owns one 2KB PSUM bank and the three sigmoids are one contiguous
    span [0:1536] handled by a single ACT instruction.
  - matmul: out[gates, batch] = W[k, gates].T @ hT[k, batch]; weights stationary
    (lhsT), state streaming; fp8e4 operands with MatmulPerfMode.DoubleRow (two
    128-row k-tiles per instruction at 0.5 cycles/row -> 4x bf16 throughput),
    fp32 PSUM accumulation.
  - Layer-0 input projection: embW = embedding @ W0x folded host-side ([80,1024])
    and applied per step as a one-hot matmul (K=81: 80 classes + a ones row
    carrying b0 + forget bias). The one-hot matrix is host-encoded from the
    int32 features (a re-encoding, no FLOPs) and kept SBUF-resident in fp8
    (0/1 values are exact).
  - Biases: each 2KB PSUM bank's accumulation group is opened by one rank-2
    fp8 matmul (lhsT rows = the bank's two gate-chunk biases, rhs =
    complementary ones/zeros rows), so all h-independent matmuls can issue
    before the h-dependent ones without two pending groups in a bank.

Pipelining (the key trick): the LSTM recurrence is a serial
matmul -> ACT -> DVE -> h chain, so a naive order stalls every step waiting
for h0(t). Layer 1 is therefore SKEWED one step behind layer 0 in program
order:
    program step t = [L0(t) matmuls][L1(t-1) matmuls]
L1(t-1) depends only on h0(t-1)/h1(t-2), so its matmuls execute while the
L0(t) activation chain produces h0(t). With fp8 matmuls the bottleneck engine
is ACT (~5.4us/step of sigmoid/tanh); the skew keeps ACT fed from two layers.

The `reps` build parameter emits the whole kernel body (DMA loads + recurrence
+ loss tail) `reps` times back-to-back in one NEFF; test.py differences a
reps=R against a reps=1 NEFF to cancel the ~80ms axon-tunnel dispatch overhead
out of the hardware-time measurement.
"""

import numpy as np
import ml_dtypes

import concourse.bass as bass
import concourse.bacc as bacc
import concourse.tile as tile
import concourse.mybir as mybir
from concourse.alu_op_type import AluOpType
from concourse.bass_utils import run_bass_kernel_spmd

AF = mybir.ActivationFunctionType
AX = mybir.AxisListType
DR = mybir.MatmulPerfMode.DoubleRow
BF16 = mybir.dt.bfloat16
F16 = mybir.dt.float16
F32 = mybir.dt.float32
F8 = mybir.dt.float8e4
MUL = AluOpType.mult
ADD = AluOpType.add

N_CORES = 8
B, T, C, E, H = 2048, 80, 80, 8, 256
BL = B // N_CORES            # 256 local batch (free dim of recurrence matmuls)
G = 4 * H                    # 1024 gate dims -> 8 chunks of 128
FB = 1.0                     # forget bias

# Gate order in reference W columns: i, j, f, o (ref slot per gate name).
_REF_SLOT = {"i": 0, "j": 1, "f": 2, "o": 3}
GATE_ORDER = "fioj"          # device gate order; sigmoid gates contiguous
C_DTYPE = F16                # cell state dtype (2-byte => DVE 2x mode)
SCHED = 2                    # 0: v1 order; 1: tanhj-first+split sig; 2: 1+defer L1 back
PIN_FIFO = True              # pin ACT/DVE order with dep edges


def _layout():
    perm = np.concatenate([np.arange(_REF_SLOT[g] * H, (_REF_SLOT[g] + 1) * H)
                           for g in GATE_ORDER])
    sl = {g: slice(i * 2 * H, (i + 1) * 2 * H) for i, g in enumerate(GATE_ORDER)}
    return perm, sl["f"], sl["i"], sl["j"], sl["o"]


def _build_nc(n_steps: int = T, reps: int = 1):
    """Build the Bass program (SPMD; same NEFF on all 8 cores)."""
    _PERM, SF, SI, SJ, SO = _layout()
    nc = bacc.Bacc("TRN2", target_bir_lowering=False, debug=False)

    d_w0h = nc.dram_tensor("w0h", [128, 2, G], F8, kind="ExternalInput").ap()
    d_w1 = nc.dram_tensor("w1", [128, 4, G], F8, kind="ExternalInput").ap()
    d_embwa = nc.dram_tensor("embwa", [C, G], F8, kind="ExternalInput").ap()
    d_oh = nc.dram_tensor("onehot", [C, n_steps * BL], F8, kind="ExternalInput").ap()
    d_b0r2 = nc.dram_tensor("b0r2", [2, 512], F8, kind="ExternalInput").ap()
    d_b1r2 = nc.dram_tensor("b1r2", [2, 512], F8, kind="ExternalInput").ap()
    d_ones_b2 = nc.dram_tensor("ones_b2", [2, 512], F8, kind="ExternalInput").ap()
    d_wd = nc.dram_tensor("wd", [128, 2, C], F8, kind="ExternalInput").ap()
    d_bd = nc.dram_tensor("bdrow", [1, C], F8, kind="ExternalInput").ap()
    d_ones_r = nc.dram_tensor("ones_r", [1, 128], F8, kind="ExternalInput").ap()
    d_ones_c = nc.dram_tensor("ones_c", [128, 1], F32, kind="ExternalInput").ap()
    d_lab = nc.dram_tensor("labels_f", [128, 2 * C], F32, kind="ExternalInput").ap()
    d_out = nc.dram_tensor("loss_out", [1, 1], F32, kind="ExternalOutput").ap()

    with tile.TileContext(nc) as tc:
        for rep in range(reps):
            _emit_body(nc, tc, rep, n_steps,
                       d_w0h, d_w1, d_embwa, d_oh, d_b0r2, d_b1r2, d_ones_b2,
                       d_wd, d_bd, d_ones_r, d_ones_c, d_lab, d_out,
                       SF, SI, SJ, SO)

    nc.compile()
    return nc


def _emit_body(nc, tc, rep, n_steps,
               d_w0h, d_w1, d_embwa, d_oh, d_b0r2, d_b1r2, d_ones_b2,
               d_wd, d_bd, d_ones_r, d_ones_c, d_lab, d_out,
               SF, SI, SJ, SO):
    R = f"r{rep}_"
    with tc.tile_pool(name=R + "consts", bufs=1) as cpool, \
         tc.tile_pool(name=R + "states", bufs=1) as spool, \
         tc.tile_pool(name=R + "gates", bufs=2) as gpool, \
         tc.tile_pool(name=R + "scratch", bufs=3) as scpool:

        w0h = cpool.tile([128, 2, G], F8)
        nc.sync.dma_start(w0h[:], d_w0h)
        w1 = cpool.tile([128, 4, G], F8)
        nc.sync.dma_start(w1[:], d_w1)
        embwa = cpool.tile([C, G], F8)
        nc.sync.dma_start(embwa[:], d_embwa)
        b0r2 = cpool.tile([2, 512], F8)
        nc.sync.dma_start(b0r2[:], d_b0r2)
        b1r2 = cpool.tile([2, 512], F8)
        nc.sync.dma_start(b1r2[:], d_b1r2)
        ones_b2 = cpool.tile([2, 512], F8)
        nc.sync.dma_start(ones_b2[:], d_ones_b2)
        wd = cpool.tile([128, 2, C], F8)
        nc.sync.dma_start(wd[:], d_wd)
        bdrow = cpool.tile([1, C], F8)
        nc.sync.dma_start(bdrow[:], d_bd)
        ones_r = cpool.tile([1, 128], F8)
        nc.sync.dma_start(ones_r[:], d_ones_r)
        ones_c = cpool.tile([128, 1], F32)
        nc.sync.dma_start(ones_c[:], d_ones_c)
        lab = cpool.tile([128, 2 * C], F32)
        nc.sync.dma_start(lab[:], d_lab)
        oh = cpool.tile([C, n_steps * BL], F8)
        n_oh_chunks = max(1, min(4, n_steps))
        csz = (n_steps * BL) // n_oh_chunks
        for i in range(n_oh_chunks):
            sl = slice(i * csz, (i + 1) * csz if i < n_oh_chunks - 1 else n_steps * BL)
            nc.sync.dma_start(oh[:, sl], d_oh[:, sl])

        # recurrent state (double-buffered h for WAR-free pipelining)
        h0t = [spool.tile([128, 2, BL], F8, name=f"{R}h0_{p}") for p in range(2)]
        h1t = [spool.tile([128, 2, BL], F8, name=f"{R}h1_{p}") for p in range(2)]
        c0 = spool.tile([128, 2 * BL], C_DTYPE)
        c1 = spool.tile([128, 2 * BL], C_DTYPE)

        # PSUM accumulation groups are per 2KB zero-region = one bank = two
        # gate chunks. Each bank's group is opened by ONE rank-2 bias matmul
        # spanning the full bank (lhsT rows = the two chunks' biases, rhs =
        # complementary ones/zeros rows), so the h-independent matmuls can
        # all be emitted before the h-dependent ones without ever having two
        # pending groups in a bank.
        def _bank_bias(ps, brow):
            for b in range(4):
                nc.tensor.matmul(ps[:, b * 512:(b + 1) * 512],
                                 brow[:, b * 128:(b + 1) * 128],
                                 ones_b2[:, :], start=True, stop=False)

        def l0_mms(t, ps, h0_prev):
            """L0 gates: bank bias (b0) + one-hot embW, then W0h DoubleRow.
            One-hot matmuls are h-independent and issue while h0(t-1) is
            still being produced."""
            oh_rhs = oh[:, t * BL:(t + 1) * BL]
            _bank_bias(ps, b0r2)
            for g in range(8):
                psg = ps[:, g * BL:(g + 1) * BL]
                gs = slice(g * 128, (g + 1) * 128)
                nc.tensor.matmul(psg, embwa[:, gs], oh_rhs, start=False,
                                 stop=(t == 0 and g % 2 == 1))
            if t > 0:
                for g in range(8):
                    psg = ps[:, g * BL:(g + 1) * BL]
                    gs = slice(g * 128, (g + 1) * 128)
                    nc.tensor.matmul(psg, w0h[:, :, gs], h0_prev[:, :, :],
                                     start=False, stop=(g % 2 == 1), perf_mode=DR)

        def l1_mms_a(s, ps, h0_s):
            """L1 gates part 1: bank bias (b1) + W1h0 (both h1-independent)."""
            last = s == 0   # no h1 part at s=0 (h1(-1) = 0)
            _bank_bias(ps, b1r2)
            for g in range(8):
                psg = ps[:, g * BL:(g + 1) * BL]
                gs = slice(g * 128, (g + 1) * 128)
                nc.tensor.matmul(psg, w1[:, 0:2, gs], h0_s[:, :, :],
                                 start=False, stop=(last and g % 2 == 1),
                                 perf_mode=DR)

        def l1_mms_b(s, ps, h1_prev):
            """L1 gates part 2: W1h1 @ h1(s-1) (emitted late: h1(s-1) is
            computed mid-step when the L1 tail is deferred). The j-gate
            chunks (6,7) go first so tanh(j1) can start before sig1's
            operands are complete."""
            for g in (6, 7, 0, 1, 2, 3, 4, 5):
                psg = ps[:, g * BL:(g + 1) * BL]
                gs = slice(g * 128, (g + 1) * 128)
                nc.tensor.matmul(psg, w1[:, 2:4, gs], h1_prev[:, :, :],
                                 start=False, stop=(g % 2 == 1), perf_mode=DR)

        # Pin ACT/DVE execution order to emission order (the Tile
        # scheduler otherwise reorders these and serializes the loop).
        _last = {"act": None, "dve": None}

        def _pin(kind, bi):
            if not PIN_FIFO:
                return bi
            if _last[kind] is not None:
                tile.add_dep_helper(bi.ins, _last[kind].ins, sync=False,
                                    reason="fifo-pin")
            _last[kind] = bi
            return bi

        def _act(*args, **kw):
            return _pin("act", nc.scalar.activation(*args, **kw))

        def _dve_tt(*args):
            return _pin("dve", nc.vector.tensor_tensor(*args))

        def front_a(t, name, ps, gsb, c, split_o=False):
            """sigmoids + start of the c chain. With split_o the o-gate
            sigmoid is emitted separately (front_o) so the f,i sigmoid — and
            with it the c chain — retires ~430ns earlier."""
            hi = 1024 if split_o else 1536
            _act(gsb[:, 0:hi], ps[:, 0:hi], AF.Sigmoid)
            if t > 0:
                _dve_tt(c[:], c[:], gsb[:, SF], MUL)

        def front_o(t, name, ps, gsb):
            _act(gsb[:, SO], ps[:, SO], AF.Sigmoid)

        def front_tanhj(t, name, ps, gsb):
            _act(gsb[:, SJ], ps[:, SJ], AF.Tanh)

        def front_m(t, name, gsb, c):
            """the i*j product and c accumulation."""
            if t == 0:
                _dve_tt(c[:], gsb[:, SI], gsb[:, SJ], MUL)
            else:
                m = scpool.tile([128, 2 * BL], BF16, tag="m", name=f"{R}m_{name}_{t}")
                _dve_tt(m[:], gsb[:, SI], gsb[:, SJ], MUL)
                _dve_tt(c[:], c[:], m[:], ADD)

        def front_b(t, name, ps, gsb, c):
            """tanh(j) + the i*j product and c accumulation."""
            front_tanhj(t, name, ps, gsb)
            front_m(t, name, gsb, c)

        def gate_back(t, name, gsb, c, h_cur):
            """tanh(c) and h = tanh(c)*sig(o) for one layer."""
            tcn = scpool.tile([128, 2 * BL], BF16, tag="tc", name=f"{R}tc_{name}_{t}")
            _act(tcn[:], c[:], AF.Tanh)
            _dve_tt(h_cur[:, :, :], tcn[:], gsb[:, SO], MUL)

        with tc.tile_pool(name=R + "psum_main", bufs=1, space="PSUM") as pmain:
            l0ps = pmain.tile([128, 2048], F32)
            l1ps = pmain.tile([128, 2048], F32)
            g1_hist = {}

            if SCHED == 0:
                # program step t: [L0(t) MMs][L1(t-1) MMs], then the L0(t)
                # chain (through tanh(c0)/h0 — the recurrence-critical path),
                # then the L1(t-1) chain.
                for t in range(n_steps + 1):
                    s = t - 1
                    if t < n_steps:
                        l0_mms(t, l0ps, h0t[(t + 1) % 2])
                    if 1 <= t:
                        l1_mms_a(s, l1ps, h0t[s % 2])
                        if s > 0:
                            l1_mms_b(s, l1ps, h1t[(s + 1) % 2])

                    if t < n_steps:
                        g0 = gpool.tile([128, 2048], BF16, tag="g0", name=f"{R}g0_{t}")
                        front_a(t, "l0", l0ps, g0, c0)
                        front_b(t, "l0", l0ps, g0, c0)
                        gate_back(t, "l0", g0, c0, h0t[t % 2])
                    if 1 <= t:
                        g1 = gpool.tile([128, 2048], BF16, tag="g1", name=f"{R}g1_{t}")
                        front_a(s, "l1", l1ps, g1, c1)
                        front_b(s, "l1", l1ps, g1, c1)
                        gate_back(s, "l1", g1, c1, h1t[s % 2])
            else:
                # SCHED 2 — L1's tail (tanh(c1), h1) deferred one step and
                # interleaved into the ACT holes of L0's chain. Program step
                # t handles: L0(t) full, L1(t-1) front, L1(t-2) back. The
                # steady-state ACT order is
                #   sig0(t), tanhc1(t-2), tanhj0(t), tanhc0(t), sig1(s), tanhj1(s)
                # with the W1h1 matmuls for step s emitted right after h1(t-2)
                # so sig1(s) finds its PSUM ready in its slot.
                for t in range(n_steps + 2):
                    s, sb = t - 1, t - 2
                    if t < n_steps:
                        l0_mms(t, l0ps, h0t[(t + 1) % 2])
                    if 1 <= t <= n_steps:
                        l1_mms_a(s, l1ps, h0t[s % 2])

                    if t < n_steps:
                        g0 = gpool.tile([128, 2048], BF16, tag="g0", name=f"{R}g0_{t}")
                        front_a(t, "l0", l0ps, g0, c0, split_o=True)
                    if 2 <= t <= n_steps + 1:
                        gate_back(sb, "l1", g1_hist.pop(sb), c1, h1t[sb % 2])
                        if sb + 1 <= n_steps - 1:
                            l1_mms_b(sb + 1, l1ps, h1t[sb % 2])
                    if t < n_steps:
                        front_b(t, "l0", l0ps, g0, c0)
                        front_o(t, "l0", l0ps, g0)
                        gate_back(t, "l0", g0, c0, h0t[t % 2])
                    if 1 <= t <= n_steps:
                        g1 = gpool.tile([128, 2048], BF16, tag="g1", name=f"{R}g1_{t}")
                        front_a(s, "l1", l1ps, g1, c1)
                        front_b(s, "l1", l1ps, g1, c1)
                        g1_hist[s] = g1

        # ---- loss tail: pred = h1.T @ Wd + bd ; softmax CE; partial sum
        h1f = h1t[(n_steps - 1) % 2]
        with tc.tile_pool(name=R + "psum_tail", bufs=1, space="PSUM") as ptail:
            losses = scpool.tile([128, 2], F32, tag="losses")
            for cidx in range(2):
                pred = ptail.tile([128, C], F32, name=f"{R}pred_{cidx}")
                bs = cidx * 128
                nc.tensor.matmul(pred[:], h1f[:, :, bs:bs + 128], wd[:, :, :],
                                 start=True, stop=False, perf_mode=DR)
                nc.tensor.matmul(pred[:], ones_r[0:1, :], bdrow[0:1, :],
                                 start=False, stop=True)
                rmax = scpool.tile([128, 1], F32, tag="r1", name=f"{R}rmax_{cidx}")
                nc.vector.reduce_max(rmax[:], pred[:], axis=AX.X)
                negmax = scpool.tile([128, 1], F32, tag="r2", name=f"{R}negmax_{cidx}")
                nc.vector.tensor_scalar_mul(negmax[:], rmax[:], -1.0)
                expt = scpool.tile([128, C], F32, tag="rC", name=f"{R}expt_{cidx}")
                sumexp = scpool.tile([128, 1], F32, tag="r3", name=f"{R}sumexp_{cidx}")
                nc.scalar.activation(expt[:], pred[:], AF.Exp,
                                     bias=negmax[:], scale=1.0)
                nc.vector.reduce_sum(sumexp[:], expt[:], axis=AX.X)
                lnz = scpool.tile([128, 1], F32, tag="r4", name=f"{R}lnz_{cidx}")
                nc.scalar.activation(lnz[:], sumexp[:], AF.Ln)
                scr = scpool.tile([128, C], F32, tag="rC2", name=f"{R}scr_{cidx}")
                dotc = scpool.tile([128, 1], F32, tag="r5", name=f"{R}dot_{cidx}")
                nc.vector.tensor_tensor(scr[:], pred[:],
                                        lab[:, cidx * C:(cidx + 1) * C], MUL)
                nc.vector.reduce_sum(dotc[:], scr[:], axis=AX.X)
                # loss = lnz - dot - negmax   (= logZ_shift - (pred[label]-max))
                nc.vector.tensor_sub(losses[:, cidx:cidx + 1], lnz[:], dotc[:])
                nc.vector.tensor_sub(losses[:, cidx:cidx + 1],
                                     losses[:, cidx:cidx + 1], negmax[:])
            total = scpool.tile([128, 1], F32, tag="r6")
            nc.vector.reduce_sum(total[:], losses[:], axis=AX.X)
            lossps = ptail.tile([1, 1], F32)
            nc.tensor.matmul(lossps[0:1, 0:1], total[:], ones_c[:], start=True, stop=True)
            out_sb = scpool.tile([1, 1], F32, tag="r7")
            nc.vector.tensor_copy(out_sb[:], lossps[0:1, 0:1])
            nc.sync.dma_start(d_out, out_sb[:])


def _prep_in_maps(inputs, n_steps: int = T):
    """Host-side input reformatting (weight packing / one-hot encoding only)."""
    feats = np.asarray(inputs["features"])
    labels = np.asarray(inputs["labels"]).astype(np.float32)
    embedding = np.asarray(inputs["embedding"], np.float32)
    W0 = np.asarray(inputs["W0"], np.float32)
    b0 = np.asarray(inputs["b0"], np.float32)
    W1 = np.asarray(inputs["W1"], np.float32)
    b1 = np.asarray(inputs["b1"], np.float32)
    Wd = np.asarray(inputs["Wd"], np.float32)
    bd = np.asarray(inputs["bd"], np.float32)

    f8 = ml_dtypes.float8_e4m3
    _PERM, _, _, _, _ = _layout()
    fmask = np.zeros(G, np.float32)
    fstart = GATE_ORDER.index("f") * H
    fmask[fstart:fstart + H] = FB

    W0x, W0h = W0[:E], W0[E:]
    W0hp = W0h[:, _PERM]                                # [256, 1024]
    w0h_host = W0hp.reshape(2, 128, G).transpose(1, 0, 2).astype(f8)

    embW = (embedding @ W0x)[:, _PERM]
    embwa_host = embW.astype(f8)

    W1p = W1[:, _PERM]                                  # [512, 1024]
    w1_host = W1p.reshape(4, 128, G).transpose(1, 0, 2).astype(f8)

    # rank-2 bank-bias operands: lhsT row r holds the bias for gate chunk
    # 2b+r of bank b at cols [b*128:(b+1)*128]; rhs ones_b2 row r is ones on
    # batch half r of the bank.
    def _bias_r2(bvec):
        bp = bvec.reshape(8, 128)                       # [chunk, 128]
        out = np.zeros((2, 512), np.float32)
        for b in range(4):
            out[0, b * 128:(b + 1) * 128] = bp[2 * b]
            out[1, b * 128:(b + 1) * 128] = bp[2 * b + 1]
        return out.astype(f8)

    b0r2_host = _bias_r2(b0[_PERM] + fmask)
    b1r2_host = _bias_r2(b1[_PERM] + fmask)
    ones_b2 = np.zeros((2, 512), np.float32)
    ones_b2[0, 0:256] = 1.0
    ones_b2[1, 256:512] = 1.0
    ones_b2 = ones_b2.astype(f8)

    wd_host = Wd.reshape(2, 128, C).transpose(1, 0, 2).astype(f8)
    bd_host = bd[None, :].astype(f8)
    ones_r = np.ones((1, 128), f8)
    ones_c = np.ones((128, 1), np.float32)

    feats = np.clip(feats, 0, C - 1)
    in_maps = []
    for core in range(N_CORES):
        fl = feats[core * BL:(core + 1) * BL, :n_steps]        # [BL, n_steps]
        oh = (fl.T[None, :, :] == np.arange(C)[:, None, None])  # [C, n_steps, BL]
        oh_host = oh.reshape(C, n_steps * BL).astype(f8)
        ll = labels[core * BL:(core + 1) * BL]                 # [BL, C]
        lab_host = np.concatenate([ll[0:128], ll[128:256]], axis=1).astype(np.float32)
        in_maps.append({
            "w0h": w0h_host, "w1": w1_host, "embwa": embwa_host,
            "onehot": oh_host, "b0r2": b0r2_host, "b1r2": b1r2_host,
            "ones_b2": ones_b2, "wd": wd_host, "bdrow": bd_host,
            "ones_r": ones_r, "ones_c": ones_c, "labels_f": lab_host,
        })
    return in_maps


_NC_CACHE = {}


def kernel_impl(inputs, n_steps: int = T, **run_kwargs):
    if n_steps not in _NC_CACHE:
        _NC_CACHE[n_steps] = _build_nc(n_steps)
    nc = _NC_CACHE[n_steps]
    in_maps = _prep_in_maps(inputs, n_steps)
    res = run_bass_kernel_spmd(nc, in_maps, core_ids=list(range(N_CORES)), **run_kwargs)
    partial = sum(float(r["loss_out"][0, 0]) for r in res.results)
    return np.float32(partial / B), res


def kernel(**inputs) -> np.ndarray:
    loss, _ = kernel_impl(inputs)
    return loss
